# revision 13
# baseline (speedup 1.0000x reference)
"""Trainium2 Bass kernel for nn_DelocalizedEmbedSparse (segment_reduce).

Math (N=131072 atoms, G=2048 graphs, F=256):
    psi in [0,1)  =>  psi // inf == 0 always  =>  k = k_table[0], v = v_table[0]
    q·k = e_Z @ (W_q @ k0)          (the NxFxF matmul collapses to a mat-vec)
    y = softplus(q·k / sqrt(F));  denom_g = segment_sum(y);  a = psi_g * y / denom_g
    out = x + silu(silu(x) @ W1) @ W2,  x = outer(a, v0)

Sharding: data-parallel over graphs — 256 contiguous graphs per core, atoms
split at graph boundaries (no cross-core communication), padded to a fixed
per-core shape.  Small weights are folded/packed on the host and replicated.

Device pipeline per core:
  P1: stream e_Z^T (bf16), s = e_Z·w via PE (M=1 matmuls), softplus on ACT,
      y chunks -> DRAM.
  P2: segment machinery without gather/scatter loops: inclusive cumsum of y
      (DVE scan along free dim + strict-lower-triangular matmul for the
      cross-partition carry), boundary gathers via indirect DMA, per-graph
      val = psi/denom, scatter +val/-val at graph starts/ends, second cumsum
      expands val back to atoms, a = y * val_expanded.  'a' is produced both
      as a flat row (ACT broadcast source) and column-transposed (PE
      transpose) for per-m-tile scalars.
  P3: MLP. silu(x)^T built directly by ACT (in = a broadcast via K=1 matmul,
      scale = v0 per-partition); layer 1 in transposed mode (lhsT = W1);
      layer 2 in natural mode (lhsT = silu(h1)^T tile); final
      out = v0*a + h2 fused in one DVE scalar_tensor_tensor from PSUM.
"""

import os
import sys

import numpy as np
import ml_dtypes

for _p in ("/opt/trn_rl_repo", "/root/.axon_site/_ro/trn_rl_repo"):
    if os.path.isdir(_p) and _p not in sys.path:
        sys.path.append(_p)

BF16 = ml_dtypes.bfloat16

N_FULL, G_FULL, F = 131072, 2048, 256
NCORES = 8
GPC = G_FULL // NCORES          # graphs per core (256)
GJ = GPC // 128                 # 2: graph packing columns


class Cfg:
    def __init__(self, C1, P2, SC, A3):
        self.C1 = C1                    # free-dim length in scalar-stage tiles
        self.P2 = P2                    # partitions in part-2 tile
        self.N1 = 128 * C1
        self.N2 = P2 * C1
        self.NPAD = self.N1 + self.N2   # padded atoms per core
        self.T2 = self.N2 // 128        # extra m-tiles from part 2
        self.NT = self.NPAD // 128      # total 128-atom m-tiles
        self.SC = SC                    # phase-1 s chunk (<=512)
        self.A3 = A3                    # phase-3 atom block (mult of 128, <=1024)
        self.NZ = 128 * ((self.NPAD + 1 + GPC + 127) // 128)  # scratch len
        self.TRASH0 = self.NPAD + 1
        assert self.NPAD % SC == 0 and self.NPAD % A3 == 0
        assert self.N2 % 128 == 0 and A3 % 128 == 0 and SC <= 512


FULL = Cfg(C1=128, P2=8, SC=512, A3=1024)
TINY = Cfg(C1=16, P2=8, SC=128, A3=128)


def build_bass(cfg):
    import concourse.bass as bass
    import concourse.bacc as bacc
    import concourse.tile as tile
    import concourse.mybir as mybir

    dt = mybir.dt
    f32, bf16, i32 = dt.float32, dt.bfloat16, dt.int32
    AF = mybir.ActivationFunctionType
    OP = mybir.AluOpType
    C1, P2, N1, N2, NPAD = cfg.C1, cfg.P2, cfg.N1, cfg.N2, cfg.NPAD
    T2, NT, SC, A3, NZ = cfg.T2, cfg.NT, cfg.SC, cfg.A3, cfg.NZ

    nc = bacc.Bacc()

    ezt_i = nc.dram_tensor("ezt", [128, 2, NPAD], bf16, kind="ExternalInput")
    psi_i = nc.dram_tensor("psig", [128, GJ], f32, kind="ExternalInput")
    posp_i = nc.dram_tensor("posp", [128, GJ], i32, kind="ExternalInput")
    posm_i = nc.dram_tensor("posm", [128, GJ], i32, kind="ExternalInput")
    wv_i = nc.dram_tensor("wv", [128, 2], bf16, kind="ExternalInput")
    w1_i = nc.dram_tensor("w1", [128, 2, F], bf16, kind="ExternalInput")
    w2_i = nc.dram_tensor("w2", [128, 2, F], bf16, kind="ExternalInput")
    vcol_i = nc.dram_tensor("vcol", [128, 2], f32, kind="ExternalInput")
    vrep_i = nc.dram_tensor("vrep", [128, F], f32, kind="ExternalInput")
    ltri_i = nc.dram_tensor("ltri", [128, 128], f32, kind="ExternalInput")
    ident_i = nc.dram_tensor("ident", [128, 128], f32, kind="ExternalInput")
    out_d = nc.dram_tensor("out", [NPAD, F], f32, kind="ExternalOutput")

    with tile.TileContext(nc) as tc:
        with (
            tc.tile_pool(name="consts", bufs=1) as cp,
            tc.tile_pool(name="dram", bufs=1, space="DRAM") as dp,
        ):
            y_d = dp.tile([NPAD], f32)
            z_d = dp.tile([NZ], f32)
            dp_d = dp.tile([NZ], f32)
            dm_d = dp.tile([NZ], f32)
            a_d = dp.tile([NPAD], f32)

            def cload(shape, dtype, src, tag):
                t = cp.tile(shape, dtype, tag=tag)
                nc.sync.dma_start(out=t[:], in_=src[:])
                return t

            w_sb = cload([128, 2], bf16, wv_i, "c_wv")
            w1_sb = cload([128, 2, F], bf16, w1_i, "c_w1")
            w2_sb = cload([128, 2, F], bf16, w2_i, "c_w2")
            vcol_sb = cload([128, 2], f32, vcol_i, "c_vcol")
            vrep_sb = cload([128, F], f32, vrep_i, "c_vrep")
            ltri_sb = cload([128, 128], f32, ltri_i, "c_ltri")
            ident_sb = cload([128, 128], f32, ident_i, "c_ident")
            psi_sb = cload([128, GJ], f32, psi_i, "c_psi")
            posp_sb = cload([128, GJ], i32, posp_i, "c_posp")
            posm_sb = cload([128, GJ], i32, posm_i, "c_posm")

            ones_r = cp.tile([1, 128], f32)
            nc.vector.memset(ones_r[:], 1.0)
            ones_m = cp.tile([128, P2], f32)
            nc.vector.memset(ones_m[:], 1.0)
            zero_sb = cp.tile([128, NZ // 128], f32)
            nc.vector.memset(zero_sb[:], 0.0)
            a_row = cp.tile([1, NPAD], f32)
            a_colT = cp.tile([128, NT], f32)

            # ---------------- phase 1: s = e_Z . w ----------------
            with (
                tc.tile_pool(name="p1", bufs=8) as p1,
                tc.tile_pool(name="p1ps", bufs=2, space="PSUM") as p1ps,
                tc.tile_pool(name="p1y", bufs=8) as p1y,
            ):
                for i in range(NPAD // SC):
                    ez_t = p1.tile([128, 2, SC], bf16, tag="ez")
                    nc.sync.dma_start(out=ez_t[:], in_=ezt_i[:, :, i * SC:(i + 1) * SC])
                    s_ps = p1ps.tile([1, SC], f32, tag="sps")
                    nc.tensor.matmul(out=s_ps[:], lhsT=w_sb[:, 0:1], rhs=ez_t[:, 0, :],
                                     start=True, stop=False)
                    nc.tensor.matmul(out=s_ps[:], lhsT=w_sb[:, 1:2], rhs=ez_t[:, 1, :],
                                     start=False, stop=True)
                    s_row = p1y.tile([1, SC], f32, tag="srow")
                    nc.scalar.copy(out=s_row[:], in_=s_ps[:])
                    nc.sync.dma_start(
                        out=y_d[i * SC:(i + 1) * SC].rearrange("(a b) -> a b", a=1),
                        in_=s_row[:])

            # ---------------- phase 2: segment machinery ----------------
            with (
                tc.tile_pool(name="scal", bufs=1) as sp,
                tc.tile_pool(name="sps2", bufs=1, space="PSUM") as sps,
            ):
                # collapse the 34 phase-1 DMA-queue waits into one barrier —
                # a DMA descriptor only carries a few sync-wait slots.
                tc.strict_bb_all_engine_barrier()
                # zero scratch
                nc.sync.dma_start(out=z_d[:].rearrange("(p c) -> p c", p=128), in_=zero_sb[:])
                nc.sync.dma_start(out=dp_d[:].rearrange("(p c) -> p c", p=128), in_=zero_sb[:])
                nc.sync.dma_start(out=dm_d[:].rearrange("(p c) -> p c", p=128), in_=zero_sb[:])

                y1 = sp.tile([128, C1], f32)
                nc.sync.dma_start(out=y1[:], in_=y_d[0:N1].rearrange("(p c) -> p c", c=C1))
                y2 = sp.tile([P2, C1], f32)
                nc.sync.dma_start(out=y2[:], in_=y_d[N1:NPAD].rearrange("(p c) -> p c", c=C1))
                # softplus(s) = ln(exp(s) + 1): the ACT tables in this toolchain
                # have no softplus entry, but ln+exp share one table set.
                nc.scalar.activation(out=y1[:], in_=y1[:], func=AF.Exp)
                nc.scalar.activation(out=y2[:], in_=y2[:], func=AF.Exp)
                nc.scalar.activation(out=y1[:], in_=y1[:], func=AF.Ln, bias=1.0)
                nc.scalar.activation(out=y2[:], in_=y2[:], func=AF.Ln, bias=1.0)

                def cumsum_pair(t1, t2, name):
                    # inclusive cumsum over atoms laid out [128,C1] then [P2,C1]
                    z1 = sp.tile([128, C1], f32, tag=name + "z1")
                    nc.vector.tensor_tensor_scan(out=z1[:], data0=t1[:], data1=t1[:],
                                                 initial=0.0, op0=OP.add, op1=OP.bypass)
                    z2 = sp.tile([P2, C1], f32, tag=name + "z2")
                    nc.vector.tensor_tensor_scan(out=z2[:], data0=t2[:], data1=t2[:],
                                                 initial=0.0, op0=OP.add, op1=OP.bypass)
                    c1_ps = sps.tile([128, 1], f32, tag=name + "c1")
                    nc.tensor.matmul(out=c1_ps[:], lhsT=ltri_sb[:], rhs=z1[:, C1 - 1:C1],
                                     start=True, stop=True)
                    c2_ps = sps.tile([P2, 1], f32, tag=name + "c2")
                    nc.tensor.matmul(out=c2_ps[:], lhsT=ones_m[:, 0:P2], rhs=z1[:, C1 - 1:C1],
                                     start=True, stop=False)
                    nc.tensor.matmul(out=c2_ps[:], lhsT=ltri_sb[0:P2, 0:P2], rhs=z2[:, C1 - 1:C1],
                                     start=False, stop=True)
                    c1s = sp.tile([128, 1], f32, tag=name + "c1s")
                    nc.vector.tensor_copy(out=c1s[:], in_=c1_ps[:])
                    c2s = sp.tile([P2, 1], f32, tag=name + "c2s")
                    nc.vector.tensor_copy(out=c2s[:], in_=c2_ps[:])
                    zf1 = sp.tile([128, C1], f32, tag=name + "zf1")
                    nc.vector.tensor_scalar_add(out=zf1[:], in0=z1[:], scalar1=c1s[:])
                    zf2 = sp.tile([P2, C1], f32, tag=name + "zf2")
                    nc.vector.tensor_scalar_add(out=zf2[:], in0=z2[:], scalar1=c2s[:])
                    return zf1, zf2

                zf1, zf2 = cumsum_pair(y1, y2, "zy")
                nc.sync.dma_start(out=z_d[1:1 + N1].rearrange("(p c) -> p c", c=C1), in_=zf1[:])
                nc.sync.dma_start(out=z_d[1 + N1:1 + NPAD].rearrange("(p c) -> p c", c=C1), in_=zf2[:])

                import concourse.bass as bass_mod
                zdv = z_d[:].rearrange("(n o) -> n o", o=1)
                zp = sp.tile([128, GJ], f32)
                zm = sp.tile([128, GJ], f32)
                for j in range(GJ):
                    nc.gpsimd.indirect_dma_start(
                        out=zp[:, j:j + 1], out_offset=None, in_=zdv,
                        in_offset=bass_mod.IndirectOffsetOnAxis(ap=posp_sb[:, j:j + 1], axis=0))
                    nc.gpsimd.indirect_dma_start(
                        out=zm[:, j:j + 1], out_offset=None, in_=zdv,
                        in_offset=bass_mod.IndirectOffsetOnAxis(ap=posm_sb[:, j:j + 1], axis=0))

                den = sp.tile([128, GJ], f32)
                nc.vector.tensor_sub(den[:], zm[:], zp[:])
                nc.vector.tensor_scalar_max(out=den[:], in0=den[:], scalar1=1e-30)
                rec = sp.tile([128, GJ], f32)
                nc.vector.reciprocal(out=rec[:], in_=den[:])
                val = sp.tile([128, GJ], f32)
                nc.vector.tensor_mul(val[:], rec[:], psi_sb[:])

                dpv = dp_d[:].rearrange("(n o) -> n o", o=1)
                dmv = dm_d[:].rearrange("(n o) -> n o", o=1)
                for j in range(GJ):
                    nc.gpsimd.indirect_dma_start(
                        out=dpv, out_offset=bass_mod.IndirectOffsetOnAxis(ap=posp_sb[:, j:j + 1], axis=0),
                        in_=val[:, j:j + 1], in_offset=None)
                    nc.gpsimd.indirect_dma_start(
                        out=dmv, out_offset=bass_mod.IndirectOffsetOnAxis(ap=posm_sb[:, j:j + 1], axis=0),
                        in_=val[:, j:j + 1], in_offset=None)

                dd1 = sp.tile([128, C1], f32)
                dd2 = sp.tile([P2, C1], f32)
                tmp1 = sp.tile([128, C1], f32)
                tmp2 = sp.tile([P2, C1], f32)
                nc.sync.dma_start(out=dd1[:], in_=dp_d[0:N1].rearrange("(p c) -> p c", c=C1))
                nc.sync.dma_start(out=dd2[:], in_=dp_d[N1:NPAD].rearrange("(p c) -> p c", c=C1))
                nc.sync.dma_start(out=tmp1[:], in_=dm_d[0:N1].rearrange("(p c) -> p c", c=C1))
                nc.sync.dma_start(out=tmp2[:], in_=dm_d[N1:NPAD].rearrange("(p c) -> p c", c=C1))
                nc.vector.tensor_sub(dd1[:], dd1[:], tmp1[:])
                nc.vector.tensor_sub(dd2[:], dd2[:], tmp2[:])

                ef1, ef2 = cumsum_pair(dd1, dd2, "zd")
                a1 = sp.tile([128, C1], f32)
                nc.vector.tensor_mul(a1[:], y1[:], ef1[:])
                a2 = sp.tile([P2, C1], f32)
                nc.vector.tensor_mul(a2[:], y2[:], ef2[:])

                nc.sync.dma_start(out=a_d[0:N1].rearrange("(p c) -> p c", c=C1), in_=a1[:])
                nc.sync.dma_start(out=a_d[N1:NPAD].rearrange("(p c) -> p c", c=C1), in_=a2[:])
                nc.sync.dma_start(out=a_row[:], in_=a_d[:].rearrange("(a b) -> a b", a=1))

                art1 = sp.tile([C1, 128], f32)
                nc.sync.dma_start(out=art1[:], in_=a_d[0:N1].rearrange("(t q) -> t q", q=128))
                art2 = sp.tile([T2, 128], f32)
                nc.sync.dma_start(out=art2[:], in_=a_d[N1:NPAD].rearrange("(t q) -> t q", q=128))
                tp1 = sps.tile([128, C1], f32, tag="tp1")
                nc.tensor.transpose(out=tp1[:], in_=art1[:], identity=ident_sb[0:C1, 0:C1])
                tp2 = sps.tile([128, T2], f32, tag="tp2")
                nc.tensor.transpose(out=tp2[:], in_=art2[:], identity=ident_sb[0:T2, 0:T2])
                nc.vector.tensor_copy(out=a_colT[:, 0:C1], in_=tp1[:])
                nc.vector.tensor_copy(out=a_colT[:, C1:NT], in_=tp2[:])

            # ---------------- phase 3: MLP ----------------
            AB = A3 // 512 if A3 >= 512 else 1
            AS = A3 // AB                      # psum sub-width (<=512)
            with (
                tc.tile_pool(name="p3", bufs=2) as p3,
                tc.tile_pool(name="abcps", bufs=1, space="PSUM") as abcps,
                tc.tile_pool(name="h1ps", bufs=2, space="PSUM") as h1ps,
                tc.tile_pool(name="ops", bufs=2, space="PSUM") as ops_,
                tc.tile_pool(name="p3o", bufs=2) as p3o,
            ):
                for b in range(NPAD // A3):
                    a_sl = a_row[:, b * A3:(b + 1) * A3]
                    abc = abcps.tile([128, AB, AS], f32, tag="abc")
                    for u in range(AB):
                        nc.tensor.matmul(out=abc[:, u, :], lhsT=ones_r[:],
                                         rhs=a_sl[:, u * AS:(u + 1) * AS],
                                         start=True, stop=True)
                    sx = p3.tile([128, 2, A3], bf16, tag="sx")
                    for k in range(2):
                        nc.scalar.activation(
                            out=sx[:, k, :].rearrange("p (u n) -> p u n", u=AB),
                            in_=abc[:], func=AF.Silu, scale=vcol_sb[:, k:k + 1])
                    sh1 = p3.tile([128, 2, A3], bf16, tag="sh1")
                    for m in range(2):
                        h1 = h1ps.tile([128, AB, AS], f32, tag="h1")
                        for k in range(2):
                            for u in range(AB):
                                nc.tensor.matmul(
                                    out=h1[:, u, :],
                                    lhsT=w1_sb[:, k, m * 128:(m + 1) * 128],
                                    rhs=sx[:, k, u * AS:(u + 1) * AS],
                                    start=(k == 0), stop=(k == 1))
                        nc.scalar.activation(
                            out=sh1[:, m, :].rearrange("p (u n) -> p u n", u=AB),
                            in_=h1[:], func=AF.Silu)
                    osb = p3o.tile([128, A3 // 128, F], f32, tag="osb")
                    for t in range(A3 // 128):
                        o_ps = ops_.tile([128, F], f32, tag="ops")
                        nc.tensor.matmul(out=o_ps[:], lhsT=sh1[:, 0, t * 128:(t + 1) * 128],
                                         rhs=w2_sb[:, 0, :], start=True, stop=False)
                        nc.tensor.matmul(out=o_ps[:], lhsT=sh1[:, 1, t * 128:(t + 1) * 128],
                                         rhs=w2_sb[:, 1, :], start=False, stop=True)
                        gt = b * (A3 // 128) + t
                        nc.vector.scalar_tensor_tensor(
                            out=osb[:, t, :], in0=vrep_sb[:], scalar=a_colT[:, gt:gt + 1],
                            in1=o_ps[:], op0=OP.mult, op1=OP.add)
                    nc.sync.dma_start(
                        out=out_d[b * A3:(b + 1) * A3, :].rearrange("(t p) f -> p t f", p=128),
                        in_=osb[:])
    nc.finalize()
    return nc


def prep_core_inputs(cfg, core, eZ, psi, gb, w_bf, w1_bf, w2_bf, v0, ltri, ident):
    """Build the per-core input map (host-side sharding + packing)."""
    NPAD, N1 = cfg.NPAD, cfg.N1
    g0 = core * GPC
    s0, e0 = int(gb[g0]), int(gb[g0 + GPC])
    n_c = e0 - s0
    assert n_c <= NPAD, f"core {core}: {n_c} atoms > NPAD {NPAD}"

    ez_c = np.zeros((NPAD, F), np.float32)
    ez_c[:n_c] = eZ[s0:e0]
    ez_pack = np.ascontiguousarray(
        ez_c.T.reshape(2, 128, NPAD).transpose(1, 0, 2)).astype(BF16)

    gl = (gb[g0:g0 + GPC + 1] - s0).astype(np.int64)
    starts, ends = gl[:-1], gl[1:]
    nonempty = ends > starts
    stt = starts[nonempty]
    end_ = ends[nonempty]
    psi_ne = psi[g0:g0 + GPC][nonempty]
    K = len(stt)
    posp = np.zeros(GPC, np.int32)
    posm = np.zeros(GPC, np.int32)
    psig = np.zeros(GPC, np.float32)
    posp[:K] = stt
    posm[:K] = end_
    psig[:K] = psi_ne
    pad = np.arange(GPC - K, dtype=np.int32)
    posp[K:] = cfg.TRASH0 + pad
    posm[K:] = cfg.TRASH0 + pad

    def pack_g(x):
        return np.ascontiguousarray(x.reshape(GJ, 128).T)

    return {
        "ezt": ez_pack,
        "psig": pack_g(psig),
        "posp": pack_g(posp),
        "posm": pack_g(posm),
        "wv": np.ascontiguousarray(w_bf.reshape(2, 128).T),
        "w1": np.ascontiguousarray(w1_bf.reshape(2, 128, F).transpose(1, 0, 2)),
        "w2": np.ascontiguousarray(w2_bf.reshape(2, 128, F).transpose(1, 0, 2)),
        "vcol": np.ascontiguousarray(v0.reshape(2, 128).T.astype(np.float32)),
        "vrep": np.ascontiguousarray(np.broadcast_to(v0, (128, F)).astype(np.float32)),
        "ltri": ltri,
        "ident": ident,
    }, (s0, e0, n_c)


_NC_CACHE = {}


def kernel(atomic_numbers, psi, batch_segments, graph_mask, e_Z,
           W_q, k_table, v_table, W_res1, W_res2):
    from concourse.bass_utils import run_bass_kernel_spmd

    cfg = FULL
    psi = np.asarray(psi, np.float32)
    seg = np.asarray(batch_segments).astype(np.int64)
    eZ = np.asarray(e_Z, np.float32).reshape(-1, F)
    N = eZ.shape[0]
    assert N == N_FULL and len(psi) == G_FULL

    # fold weights: s = e_Z @ (W_q @ k0) / sqrt(F)   (psi // inf == 0 always)
    k0 = np.asarray(k_table, np.float32)[0]
    v0 = np.asarray(v_table, np.float32)[0]
    w = (np.asarray(W_q, np.float32) @ k0) * (1.0 / np.sqrt(F))
    w_bf = w.astype(BF16)
    w1_bf = np.asarray(W_res1, np.float32).astype(BF16)
    w2_bf = np.asarray(W_res2, np.float32).astype(BF16)
    ltri = np.triu(np.ones((128, 128), np.float32), 1)
    ident = np.eye(128, dtype=np.float32)

    gb = np.searchsorted(seg, np.arange(G_FULL + 1))

    in_maps, spans = [], []
    for c in range(NCORES):
        m, span = prep_core_inputs(cfg, c, eZ, psi, gb, w_bf, w1_bf, w2_bf,
                                   v0, ltri, ident)
        in_maps.append(m)
        spans.append(span)

    if "nc" not in _NC_CACHE:
        _NC_CACHE["nc"] = build_bass(cfg)
    nc = _NC_CACHE["nc"]

    trace = os.environ.get("KERNEL_TRACE", "") == "1"
    res = run_bass_kernel_spmd(nc, in_maps, core_ids=list(range(NCORES)),
                               trace=trace)
    if trace:
        kernel.last_exec_time_ns = res.exec_time_ns
        kernel.last_results = res

    out = np.empty((N, F), np.float32)
    for c in range(NCORES):
        s0, e0, n_c = spans[c]
        out[s0:e0] = res.results[c]["out"][:n_c]
    return out.reshape(N, 1, 1, F)


# revision 21
# speedup vs baseline: 1.1957x; 1.1957x over previous
"""Trainium2 Bass kernel for nn_DelocalizedEmbedSparse (segment_reduce).

Math (N=131072 atoms, G=2048 graphs, F=256):
    psi in [0,1)  =>  psi // inf == 0 always  =>  k = k_table[0], v = v_table[0]
    q·k = e_Z @ (W_q @ k0)          (the NxFxF matmul collapses to a mat-vec)
    y = softplus(q·k / sqrt(F));  denom_g = segment_sum(y);  a = psi_g * y / denom_g
    out = x + silu(silu(x) @ W1) @ W2,  x = outer(a, v0)

Sharding: data-parallel over graphs — 256 contiguous graphs per core, atoms
split at graph boundaries (no cross-core communication), padded to a fixed
per-core shape.  Small weights are folded/packed on the host and replicated.

Device pipeline per core:
  P1: stream e_Z^T (bf16), s = e_Z·w via PE (M=1 matmuls), softplus on ACT,
      y chunks -> DRAM.
  P2: segment machinery without gather/scatter loops: inclusive cumsum of y
      (DVE scan along free dim + strict-lower-triangular matmul for the
      cross-partition carry), boundary gathers via indirect DMA, per-graph
      val = psi/denom, scatter +val/-val at graph starts/ends, second cumsum
      expands val back to atoms, a = y * val_expanded.  'a' is produced both
      as a flat row (ACT broadcast source) and column-transposed (PE
      transpose) for per-m-tile scalars.
  P3: MLP. silu(x)^T built directly by ACT (in = a broadcast via K=1 matmul,
      scale = v0 per-partition); layer 1 in transposed mode (lhsT = W1);
      layer 2 in natural mode (lhsT = silu(h1)^T tile); final
      out = v0*a + h2 fused in one DVE scalar_tensor_tensor from PSUM.
"""

import os
import sys

import numpy as np
import ml_dtypes

for _p in ("/opt/trn_rl_repo", "/root/.axon_site/_ro/trn_rl_repo"):
    if os.path.isdir(_p) and _p not in sys.path:
        sys.path.append(_p)

BF16 = ml_dtypes.bfloat16

N_FULL, G_FULL, F = 131072, 2048, 256
NCORES = 8
GPC = G_FULL // NCORES          # graphs per core (256)
GJ = GPC // 128                 # 2: graph packing columns


class Cfg:
    def __init__(self, C1, P2, SC, A3):
        self.C1 = C1                    # free-dim length in scalar-stage tiles
        self.P2 = P2                    # partitions in part-2 tile
        self.N1 = 128 * C1
        self.N2 = P2 * C1
        self.NPAD = self.N1 + self.N2   # padded atoms per core
        self.T2 = self.N2 // 128        # extra m-tiles from part 2
        self.NT = self.NPAD // 128      # total 128-atom m-tiles
        self.SC = SC                    # phase-1 s chunk (<=512)
        self.A3 = A3                    # phase-3 atom block (mult of 128, <=1024)
        self.NZ = 128 * ((self.NPAD + 1 + GPC + 127) // 128)  # scratch len
        self.TRASH0 = self.NPAD + 1
        assert self.NPAD % SC == 0 and self.NPAD % A3 == 0
        assert self.N2 % 128 == 0 and A3 % 128 == 0 and SC <= 512


FULL = Cfg(C1=128, P2=8, SC=512, A3=1024)
TINY = Cfg(C1=16, P2=8, SC=128, A3=128)


def build_bass(cfg):
    import concourse.bass as bass
    import concourse.bacc as bacc
    import concourse.tile as tile
    import concourse.mybir as mybir

    dt = mybir.dt
    f32, bf16, i32 = dt.float32, dt.bfloat16, dt.int32
    AF = mybir.ActivationFunctionType
    OP = mybir.AluOpType
    C1, P2, N1, N2, NPAD = cfg.C1, cfg.P2, cfg.N1, cfg.N2, cfg.NPAD
    T2, NT, SC, A3, NZ = cfg.T2, cfg.NT, cfg.SC, cfg.A3, cfg.NZ

    nc = bacc.Bacc()

    NB1 = NPAD // SC
    ezt_i = nc.dram_tensor("ezt", [NB1, 128, 2, SC], bf16, kind="ExternalInput")
    psi_i = nc.dram_tensor("psig", [128, GJ], f32, kind="ExternalInput")
    posp_i = nc.dram_tensor("posp", [128, GJ], i32, kind="ExternalInput")
    posm_i = nc.dram_tensor("posm", [128, GJ], i32, kind="ExternalInput")
    wv_i = nc.dram_tensor("wv", [128, 2], bf16, kind="ExternalInput")
    w1_i = nc.dram_tensor("w1", [128, 2, F], bf16, kind="ExternalInput")
    w2_i = nc.dram_tensor("w2", [128, 2, F], bf16, kind="ExternalInput")
    vcol_i = nc.dram_tensor("vcol", [128, 2], f32, kind="ExternalInput")
    vrep_i = nc.dram_tensor("vrep", [128, F], f32, kind="ExternalInput")
    ltri_i = nc.dram_tensor("ltri", [128, 128], f32, kind="ExternalInput")
    ident_i = nc.dram_tensor("ident", [128, 128], f32, kind="ExternalInput")
    out_d = nc.dram_tensor("out", [NPAD, F], f32, kind="ExternalOutput")

    with tile.TileContext(nc) as tc:
        with (
            tc.tile_pool(name="consts", bufs=1) as cp,
            tc.tile_pool(name="dram", bufs=1, space="DRAM") as dp,
        ):
            y_d = dp.tile([NPAD], f32)
            z_d = dp.tile([NZ], f32)
            dp_d = dp.tile([NZ], f32)
            dm_d = dp.tile([NZ], f32)
            a_d = dp.tile([NPAD], f32)
            ab_d = dp.tile([NPAD], bf16)

            def cload(shape, dtype, src, tag):
                t = cp.tile(shape, dtype, tag=tag)
                nc.sync.dma_start(out=t[:], in_=src[:])
                return t

            w_sb = cload([128, 2], bf16, wv_i, "c_wv")
            w1_sb = cload([128, 2, F], bf16, w1_i, "c_w1")
            w2_sb = cload([128, 2, F], bf16, w2_i, "c_w2")
            vcol_sb = cload([128, 2], f32, vcol_i, "c_vcol")
            vrep_sb = cload([128, F], f32, vrep_i, "c_vrep")
            ltri_sb = cload([128, 128], f32, ltri_i, "c_ltri")
            ident_sb = cload([128, 128], f32, ident_i, "c_ident")
            psi_sb = cload([128, GJ], f32, psi_i, "c_psi")
            posp_sb = cload([128, GJ], i32, posp_i, "c_posp")
            posm_sb = cload([128, GJ], i32, posm_i, "c_posm")

            ones_m = cp.tile([128, P2], f32)
            nc.vector.memset(ones_m[:], 1.0)
            zero_sb = cp.tile([128, NZ // 128], f32)
            nc.vector.memset(zero_sb[:], 0.0)
            a_colT = cp.tile([128, NT], f32)

            # zero DRAM scratch early — overlaps with phase 1
            nc.sync.dma_start(out=z_d[:].rearrange("(p c) -> p c", p=128), in_=zero_sb[:])
            nc.sync.dma_start(out=dp_d[:].rearrange("(p c) -> p c", p=128), in_=zero_sb[:])
            nc.sync.dma_start(out=dm_d[:].rearrange("(p c) -> p c", p=128), in_=zero_sb[:])

            # ---------------- phase 1: s = e_Z . w ----------------
            with (
                tc.tile_pool(name="p1", bufs=8) as p1,
                tc.tile_pool(name="p1ps", bufs=4, space="PSUM") as p1ps,
                tc.tile_pool(name="p1y", bufs=8) as p1y,
            ):
                for i in range(NPAD // SC):
                    ez_t = p1.tile([128, 2, SC], bf16, tag="ez")
                    nc.sync.dma_start(out=ez_t[:], in_=ezt_i[i])
                    s_ps = p1ps.tile([1, SC], f32, tag="sps")
                    nc.tensor.matmul(out=s_ps[:], lhsT=w_sb[:, 0:1], rhs=ez_t[:, 0, :],
                                     start=True, stop=False)
                    nc.tensor.matmul(out=s_ps[:], lhsT=w_sb[:, 1:2], rhs=ez_t[:, 1, :],
                                     start=False, stop=True)
                    s_row = p1y.tile([1, SC], f32, tag="srow")
                    if i % 2 == 0:
                        nc.scalar.copy(out=s_row[:], in_=s_ps[:])
                    else:
                        nc.vector.tensor_copy(out=s_row[:], in_=s_ps[:])
                    nc.sync.dma_start(
                        out=y_d[i * SC:(i + 1) * SC].rearrange("(a b) -> a b", a=1),
                        in_=s_row[:])

            # ---------------- phase 2: segment machinery ----------------
            with (
                tc.tile_pool(name="scal", bufs=1) as sp,
                tc.tile_pool(name="sps2", bufs=1, space="PSUM") as sps,
            ):
                # collapse the 34 phase-1 DMA-queue waits into one barrier —
                # a DMA descriptor only carries a few sync-wait slots.
                tc.strict_bb_all_engine_barrier()

                y1 = sp.tile([128, C1], f32)
                nc.sync.dma_start(out=y1[:], in_=y_d[0:N1].rearrange("(p c) -> p c", c=C1))
                y2 = sp.tile([P2, C1], f32)
                nc.sync.dma_start(out=y2[:], in_=y_d[N1:NPAD].rearrange("(p c) -> p c", c=C1))
                # softplus(s) = ln(exp(s) + 1): the ACT tables in this toolchain
                # have no softplus entry, but ln+exp share one table set.
                nc.scalar.activation(out=y1[:], in_=y1[:], func=AF.Exp)
                nc.scalar.activation(out=y2[:], in_=y2[:], func=AF.Exp)
                nc.scalar.activation(out=y1[:], in_=y1[:], func=AF.Ln, bias=1.0)
                nc.scalar.activation(out=y2[:], in_=y2[:], func=AF.Ln, bias=1.0)

                def cumsum_pair(t1, t2, name):
                    # inclusive cumsum over atoms laid out [128,C1] then [P2,C1]
                    z1 = sp.tile([128, C1], f32, tag=name + "z1")
                    nc.vector.tensor_tensor_scan(out=z1[:], data0=t1[:], data1=t1[:],
                                                 initial=0.0, op0=OP.add, op1=OP.bypass)
                    z2 = sp.tile([P2, C1], f32, tag=name + "z2")
                    nc.vector.tensor_tensor_scan(out=z2[:], data0=t2[:], data1=t2[:],
                                                 initial=0.0, op0=OP.add, op1=OP.bypass)
                    c1_ps = sps.tile([128, 1], f32, tag=name + "c1")
                    nc.tensor.matmul(out=c1_ps[:], lhsT=ltri_sb[:], rhs=z1[:, C1 - 1:C1],
                                     start=True, stop=True)
                    c2_ps = sps.tile([P2, 1], f32, tag=name + "c2")
                    nc.tensor.matmul(out=c2_ps[:], lhsT=ones_m[:, 0:P2], rhs=z1[:, C1 - 1:C1],
                                     start=True, stop=False)
                    nc.tensor.matmul(out=c2_ps[:], lhsT=ltri_sb[0:P2, 0:P2], rhs=z2[:, C1 - 1:C1],
                                     start=False, stop=True)
                    c1s = sp.tile([128, 1], f32, tag=name + "c1s")
                    nc.vector.tensor_copy(out=c1s[:], in_=c1_ps[:])
                    c2s = sp.tile([P2, 1], f32, tag=name + "c2s")
                    nc.vector.tensor_copy(out=c2s[:], in_=c2_ps[:])
                    zf1 = sp.tile([128, C1], f32, tag=name + "zf1")
                    nc.vector.tensor_scalar_add(out=zf1[:], in0=z1[:], scalar1=c1s[:])
                    zf2 = sp.tile([P2, C1], f32, tag=name + "zf2")
                    nc.vector.tensor_scalar_add(out=zf2[:], in0=z2[:], scalar1=c2s[:])
                    return zf1, zf2

                zf1, zf2 = cumsum_pair(y1, y2, "zy")
                nc.sync.dma_start(out=z_d[1:1 + N1].rearrange("(p c) -> p c", c=C1), in_=zf1[:])
                nc.sync.dma_start(out=z_d[1 + N1:1 + NPAD].rearrange("(p c) -> p c", c=C1), in_=zf2[:])

                import concourse.bass as bass_mod
                zdv = z_d[:].rearrange("(n o) -> n o", o=1)
                zp = sp.tile([128, GJ], f32)
                zm = sp.tile([128, GJ], f32)
                for j in range(GJ):
                    nc.gpsimd.indirect_dma_start(
                        out=zp[:, j:j + 1], out_offset=None, in_=zdv,
                        in_offset=bass_mod.IndirectOffsetOnAxis(ap=posp_sb[:, j:j + 1], axis=0))
                    nc.gpsimd.indirect_dma_start(
                        out=zm[:, j:j + 1], out_offset=None, in_=zdv,
                        in_offset=bass_mod.IndirectOffsetOnAxis(ap=posm_sb[:, j:j + 1], axis=0))

                den = sp.tile([128, GJ], f32)
                nc.vector.tensor_sub(den[:], zm[:], zp[:])
                nc.vector.tensor_scalar_max(out=den[:], in0=den[:], scalar1=1e-30)
                rec = sp.tile([128, GJ], f32)
                nc.vector.reciprocal(out=rec[:], in_=den[:])
                val = sp.tile([128, GJ], f32)
                nc.vector.tensor_mul(val[:], rec[:], psi_sb[:])

                dpv = dp_d[:].rearrange("(n o) -> n o", o=1)
                dmv = dm_d[:].rearrange("(n o) -> n o", o=1)
                for j in range(GJ):
                    nc.gpsimd.indirect_dma_start(
                        out=dpv, out_offset=bass_mod.IndirectOffsetOnAxis(ap=posp_sb[:, j:j + 1], axis=0),
                        in_=val[:, j:j + 1], in_offset=None)
                    nc.gpsimd.indirect_dma_start(
                        out=dmv, out_offset=bass_mod.IndirectOffsetOnAxis(ap=posm_sb[:, j:j + 1], axis=0),
                        in_=val[:, j:j + 1], in_offset=None)

                dd1 = sp.tile([128, C1], f32)
                dd2 = sp.tile([P2, C1], f32)
                tmp1 = sp.tile([128, C1], f32)
                tmp2 = sp.tile([P2, C1], f32)
                nc.sync.dma_start(out=dd1[:], in_=dp_d[0:N1].rearrange("(p c) -> p c", c=C1))
                nc.sync.dma_start(out=dd2[:], in_=dp_d[N1:NPAD].rearrange("(p c) -> p c", c=C1))
                nc.sync.dma_start(out=tmp1[:], in_=dm_d[0:N1].rearrange("(p c) -> p c", c=C1))
                nc.sync.dma_start(out=tmp2[:], in_=dm_d[N1:NPAD].rearrange("(p c) -> p c", c=C1))
                nc.vector.tensor_sub(dd1[:], dd1[:], tmp1[:])
                nc.vector.tensor_sub(dd2[:], dd2[:], tmp2[:])

                ef1, ef2 = cumsum_pair(dd1, dd2, "zd")
                a1 = sp.tile([128, C1], f32)
                nc.vector.tensor_mul(a1[:], y1[:], ef1[:])
                a2 = sp.tile([P2, C1], f32)
                nc.vector.tensor_mul(a2[:], y2[:], ef2[:])

                nc.sync.dma_start(out=a_d[0:N1].rearrange("(p c) -> p c", c=C1), in_=a1[:])
                nc.sync.dma_start(out=a_d[N1:NPAD].rearrange("(p c) -> p c", c=C1), in_=a2[:])
                # bf16 copy of a for the phase-3 partition-broadcast DMA
                ab1 = sp.tile([128, C1], bf16)
                nc.vector.tensor_copy(out=ab1[:], in_=a1[:])
                ab2 = sp.tile([P2, C1], bf16)
                nc.vector.tensor_copy(out=ab2[:], in_=a2[:])
                nc.sync.dma_start(out=ab_d[0:N1].rearrange("(p c) -> p c", c=C1), in_=ab1[:])
                nc.sync.dma_start(out=ab_d[N1:NPAD].rearrange("(p c) -> p c", c=C1), in_=ab2[:])

                art1 = sp.tile([C1, 128], f32)
                nc.sync.dma_start(out=art1[:], in_=a_d[0:N1].rearrange("(t q) -> t q", q=128))
                art2 = sp.tile([T2, 128], f32)
                nc.sync.dma_start(out=art2[:], in_=a_d[N1:NPAD].rearrange("(t q) -> t q", q=128))
                tp1 = sps.tile([128, C1], f32, tag="tp1")
                nc.tensor.transpose(out=tp1[:], in_=art1[:], identity=ident_sb[0:C1, 0:C1])
                tp2 = sps.tile([128, T2], f32, tag="tp2")
                nc.tensor.transpose(out=tp2[:], in_=art2[:], identity=ident_sb[0:T2, 0:T2])
                nc.vector.tensor_copy(out=a_colT[:, 0:C1], in_=tp1[:])
                nc.vector.tensor_copy(out=a_colT[:, C1:NT], in_=tp2[:])

            # ---------------- phase 3: MLP ----------------
            AB = A3 // 512 if A3 >= 512 else 1
            AS = A3 // AB                      # psum sub-width (<=512)
            with (
                tc.tile_pool(name="p3", bufs=2) as p3,
                tc.tile_pool(name="h1ps", bufs=2, space="PSUM") as h1ps,
                tc.tile_pool(name="ops", bufs=4, space="PSUM") as ops_,
                tc.tile_pool(name="p3o", bufs=2) as p3o,
            ):
                for b in range(NPAD // A3):
                    # broadcast a (bf16) down all 128 partitions via DMA
                    a_bc = p3.tile([128, A3], bf16, tag="abc")
                    a_sl = ab_d[b * A3:(b + 1) * A3]
                    a_sl_bc = bass.AP(
                        tensor=a_sl.tensor, offset=a_sl.offset,
                        ap=[[0, 128]] + [list(x) for x in a_sl.ap][-1:])
                    nc.sync.dma_start(out=a_bc[:], in_=a_sl_bc)
                    sx = p3.tile([128, 2, A3], bf16, tag="sx")
                    for k in range(2):
                        nc.scalar.activation(
                            out=sx[:, k, :], in_=a_bc[:], func=AF.Silu,
                            scale=vcol_sb[:, k:k + 1])
                    sh1 = p3.tile([128, 2, A3], bf16, tag="sh1")
                    for m in range(2):
                        h1 = h1ps.tile([128, AB, AS], f32, tag="h1")
                        for k in range(2):
                            for u in range(AB):
                                nc.tensor.matmul(
                                    out=h1[:, u, :],
                                    lhsT=w1_sb[:, k, m * 128:(m + 1) * 128],
                                    rhs=sx[:, k, u * AS:(u + 1) * AS],
                                    start=(k == 0), stop=(k == 1))
                        nc.scalar.activation(
                            out=sh1[:, m, :].rearrange("p (u n) -> p u n", u=AB),
                            in_=h1[:], func=AF.Silu)
                    osb = p3o.tile([128, A3 // 128, F], f32, tag="osb")
                    for t in range(A3 // 128):
                        o_ps = ops_.tile([128, F], f32, tag="ops")
                        nc.tensor.matmul(out=o_ps[:], lhsT=sh1[:, 0, t * 128:(t + 1) * 128],
                                         rhs=w2_sb[:, 0, :], start=True, stop=False)
                        nc.tensor.matmul(out=o_ps[:], lhsT=sh1[:, 1, t * 128:(t + 1) * 128],
                                         rhs=w2_sb[:, 1, :], start=False, stop=True)
                        gt = b * (A3 // 128) + t
                        nc.vector.scalar_tensor_tensor(
                            out=osb[:, t, :], in0=vrep_sb[:], scalar=a_colT[:, gt:gt + 1],
                            in1=o_ps[:], op0=OP.mult, op1=OP.add)
                    nc.sync.dma_start(
                        out=out_d[b * A3:(b + 1) * A3, :].rearrange("(t p) f -> p t f", p=128),
                        in_=osb[:])
    nc.finalize()
    return nc


def prep_core_inputs(cfg, core, eZ, psi, gb, w_bf, w1_bf, w2_bf, v0, ltri, ident):
    """Build the per-core input map (host-side sharding + packing)."""
    NPAD, N1 = cfg.NPAD, cfg.N1
    g0 = core * GPC
    s0, e0 = int(gb[g0]), int(gb[g0 + GPC])
    n_c = e0 - s0
    assert n_c <= NPAD, f"core {core}: {n_c} atoms > NPAD {NPAD}"

    ez_c = np.zeros((NPAD, F), np.float32)
    ez_c[:n_c] = eZ[s0:e0]
    NB1 = NPAD // cfg.SC
    ez_pack = np.ascontiguousarray(
        ez_c.reshape(NB1, cfg.SC, 2, 128).transpose(0, 3, 2, 1)).astype(BF16)

    gl = (gb[g0:g0 + GPC + 1] - s0).astype(np.int64)
    starts, ends = gl[:-1], gl[1:]
    nonempty = ends > starts
    stt = starts[nonempty]
    end_ = ends[nonempty]
    psi_ne = psi[g0:g0 + GPC][nonempty]
    K = len(stt)
    posp = np.zeros(GPC, np.int32)
    posm = np.zeros(GPC, np.int32)
    psig = np.zeros(GPC, np.float32)
    posp[:K] = stt
    posm[:K] = end_
    psig[:K] = psi_ne
    pad = np.arange(GPC - K, dtype=np.int32)
    posp[K:] = cfg.TRASH0 + pad
    posm[K:] = cfg.TRASH0 + pad

    def pack_g(x):
        return np.ascontiguousarray(x.reshape(GJ, 128).T)

    return {
        "ezt": ez_pack,
        "psig": pack_g(psig),
        "posp": pack_g(posp),
        "posm": pack_g(posm),
        "wv": np.ascontiguousarray(w_bf.reshape(2, 128).T),
        "w1": np.ascontiguousarray(w1_bf.reshape(2, 128, F).transpose(1, 0, 2)),
        "w2": np.ascontiguousarray(w2_bf.reshape(2, 128, F).transpose(1, 0, 2)),
        "vcol": np.ascontiguousarray(v0.reshape(2, 128).T.astype(np.float32)),
        "vrep": np.ascontiguousarray(np.broadcast_to(v0, (128, F)).astype(np.float32)),
        "ltri": ltri,
        "ident": ident,
    }, (s0, e0, n_c)


_NC_CACHE = {}


def kernel(atomic_numbers, psi, batch_segments, graph_mask, e_Z,
           W_q, k_table, v_table, W_res1, W_res2):
    from concourse.bass_utils import run_bass_kernel_spmd

    cfg = FULL
    psi = np.asarray(psi, np.float32)
    seg = np.asarray(batch_segments).astype(np.int64)
    eZ = np.asarray(e_Z, np.float32).reshape(-1, F)
    N = eZ.shape[0]
    assert N == N_FULL and len(psi) == G_FULL

    # fold weights: s = e_Z @ (W_q @ k0) / sqrt(F)   (psi // inf == 0 always)
    k0 = np.asarray(k_table, np.float32)[0]
    v0 = np.asarray(v_table, np.float32)[0]
    w = (np.asarray(W_q, np.float32) @ k0) * (1.0 / np.sqrt(F))
    w_bf = w.astype(BF16)
    w1_bf = np.asarray(W_res1, np.float32).astype(BF16)
    w2_bf = np.asarray(W_res2, np.float32).astype(BF16)
    ltri = np.triu(np.ones((128, 128), np.float32), 1)
    ident = np.eye(128, dtype=np.float32)

    gb = np.searchsorted(seg, np.arange(G_FULL + 1))

    in_maps, spans = [], []
    for c in range(NCORES):
        m, span = prep_core_inputs(cfg, c, eZ, psi, gb, w_bf, w1_bf, w2_bf,
                                   v0, ltri, ident)
        in_maps.append(m)
        spans.append(span)

    if "nc" not in _NC_CACHE:
        _NC_CACHE["nc"] = build_bass(cfg)
    nc = _NC_CACHE["nc"]

    trace = os.environ.get("KERNEL_TRACE", "") == "1"
    res = run_bass_kernel_spmd(nc, in_maps, core_ids=list(range(NCORES)),
                               trace=trace)
    if trace:
        kernel.last_exec_time_ns = res.exec_time_ns
        kernel.last_results = res

    out = np.empty((N, F), np.float32)
    for c in range(NCORES):
        s0, e0, n_c = spans[c]
        out[s0:e0] = res.results[c]["out"][:n_c]
    return out.reshape(N, 1, 1, F)


# revision 22
# speedup vs baseline: 1.4036x; 1.1739x over previous
"""Trainium2 Bass kernel for nn_DelocalizedEmbedSparse (segment_reduce).

Math (N=131072 atoms, G=2048 graphs, F=256):
    psi in [0,1)  =>  psi // inf == 0 always  =>  k = k_table[0], v = v_table[0]
    q·k = e_Z @ (W_q @ k0)          (the NxFxF matmul collapses to a mat-vec)
    y = softplus(q·k / sqrt(F));  denom_g = segment_sum(y);  a = psi_g * y / denom_g
    out = x + silu(silu(x) @ W1) @ W2,  x = outer(a, v0)

Sharding: data-parallel over graphs — 256 contiguous graphs per core, atoms
split at graph boundaries (no cross-core communication), padded to a fixed
per-core shape.  Small weights are folded/packed on the host and replicated.

Device pipeline per core:
  P1: stream e_Z^T (bf16), s = e_Z·w via PE (M=1 matmuls), softplus on ACT,
      y chunks -> DRAM.
  P2: segment machinery without gather/scatter loops: inclusive cumsum of y
      (DVE scan along free dim + strict-lower-triangular matmul for the
      cross-partition carry), boundary gathers via indirect DMA, per-graph
      val = psi/denom, scatter +val/-val at graph starts/ends, second cumsum
      expands val back to atoms, a = y * val_expanded.  'a' is produced both
      as a flat row (ACT broadcast source) and column-transposed (PE
      transpose) for per-m-tile scalars.
  P3: MLP. silu(x)^T built directly by ACT (in = a broadcast via K=1 matmul,
      scale = v0 per-partition); layer 1 in transposed mode (lhsT = W1);
      layer 2 in natural mode (lhsT = silu(h1)^T tile); final
      out = v0*a + h2 fused in one DVE scalar_tensor_tensor from PSUM.
"""

import os
import sys

import numpy as np
import ml_dtypes

for _p in ("/opt/trn_rl_repo", "/root/.axon_site/_ro/trn_rl_repo"):
    if os.path.isdir(_p) and _p not in sys.path:
        sys.path.append(_p)

BF16 = ml_dtypes.bfloat16

N_FULL, G_FULL, F = 131072, 2048, 256
NCORES = 8
GPC = G_FULL // NCORES          # graphs per core (256)
GJ = GPC // 128                 # 2: graph packing columns


class Cfg:
    def __init__(self, C1, P2, SC, A3):
        self.C1 = C1                    # free-dim length in scalar-stage tiles
        self.P2 = P2                    # partitions in part-2 tile
        self.N1 = 128 * C1
        self.N2 = P2 * C1
        self.NPAD = self.N1 + self.N2   # padded atoms per core
        self.T2 = self.N2 // 128        # extra m-tiles from part 2
        self.NT = self.NPAD // 128      # total 128-atom m-tiles
        self.SC = SC                    # phase-1 s chunk (<=512)
        self.A3 = A3                    # phase-3 atom block (mult of 128, <=1024)
        self.NZ = 128 * ((self.NPAD + 1 + GPC + 127) // 128)  # scratch len
        self.TRASH0 = self.NPAD + 1
        assert self.NPAD % SC == 0 and self.NPAD % A3 == 0
        assert self.N2 % 128 == 0 and A3 % 128 == 0 and SC <= 512


FULL = Cfg(C1=128, P2=8, SC=512, A3=1024)
TINY = Cfg(C1=16, P2=8, SC=128, A3=128)


def build_bass(cfg):
    import concourse.bass as bass
    import concourse.bacc as bacc
    import concourse.tile as tile
    import concourse.mybir as mybir

    dt = mybir.dt
    f32, bf16, i32 = dt.float32, dt.bfloat16, dt.int32
    AF = mybir.ActivationFunctionType
    OP = mybir.AluOpType
    C1, P2, N1, N2, NPAD = cfg.C1, cfg.P2, cfg.N1, cfg.N2, cfg.NPAD
    T2, NT, SC, A3, NZ = cfg.T2, cfg.NT, cfg.SC, cfg.A3, cfg.NZ

    nc = bacc.Bacc()

    NB1 = NPAD // SC
    ezt_i = nc.dram_tensor("ezt", [NB1, 128, 2, SC], bf16, kind="ExternalInput")
    psi_i = nc.dram_tensor("psig", [128, GJ], f32, kind="ExternalInput")
    posp_i = nc.dram_tensor("posp", [128, GJ], i32, kind="ExternalInput")
    posm_i = nc.dram_tensor("posm", [128, GJ], i32, kind="ExternalInput")
    wv_i = nc.dram_tensor("wv", [128, 2], bf16, kind="ExternalInput")
    w1_i = nc.dram_tensor("w1", [128, 2, F], bf16, kind="ExternalInput")
    w2_i = nc.dram_tensor("w2", [128, 2, F], bf16, kind="ExternalInput")
    vcol_i = nc.dram_tensor("vcol", [128, 2], f32, kind="ExternalInput")
    vrep_i = nc.dram_tensor("vrep", [128, F], f32, kind="ExternalInput")
    ltri_i = nc.dram_tensor("ltri", [128, 128], f32, kind="ExternalInput")
    ident_i = nc.dram_tensor("ident", [128, 128], f32, kind="ExternalInput")
    out_d = nc.dram_tensor("out", [NPAD, F], f32, kind="ExternalOutput")

    with tile.TileContext(nc) as tc:
        with (
            tc.tile_pool(name="consts", bufs=1) as cp,
            tc.tile_pool(name="dram", bufs=1, space="DRAM") as dp,
        ):
            y_d = dp.tile([NPAD], f32)
            z_d = dp.tile([NZ], f32)
            dp_d = dp.tile([NZ], f32)
            dm_d = dp.tile([NZ], f32)
            a_d = dp.tile([NPAD], f32)
            ab_d = dp.tile([NPAD], bf16)

            def cload(shape, dtype, src, tag):
                t = cp.tile(shape, dtype, tag=tag)
                nc.sync.dma_start(out=t[:], in_=src[:])
                return t

            w_sb = cload([128, 2], bf16, wv_i, "c_wv")
            w1_sb = cload([128, 2, F], bf16, w1_i, "c_w1")
            w2_sb = cload([128, 2, F], bf16, w2_i, "c_w2")
            vcol_sb = cload([128, 2], f32, vcol_i, "c_vcol")
            vrep_sb = cload([128, F], f32, vrep_i, "c_vrep")
            ltri_sb = cload([128, 128], f32, ltri_i, "c_ltri")
            ident_sb = cload([128, 128], f32, ident_i, "c_ident")
            psi_sb = cload([128, GJ], f32, psi_i, "c_psi")
            posp_sb = cload([128, GJ], i32, posp_i, "c_posp")
            posm_sb = cload([128, GJ], i32, posm_i, "c_posm")

            ones_m = cp.tile([128, P2], f32)
            nc.vector.memset(ones_m[:], 1.0)
            zero_sb = cp.tile([128, NZ // 128], f32)
            nc.vector.memset(zero_sb[:], 0.0)
            a_colT = cp.tile([128, NT], f32)

            # zero DRAM scratch early — overlaps with phase 1
            nc.sync.dma_start(out=z_d[:].rearrange("(p c) -> p c", p=128), in_=zero_sb[:])
            nc.sync.dma_start(out=dp_d[:].rearrange("(p c) -> p c", p=128), in_=zero_sb[:])
            nc.sync.dma_start(out=dm_d[:].rearrange("(p c) -> p c", p=128), in_=zero_sb[:])

            # ---------------- phase 1: s = e_Z . w ----------------
            with (
                tc.tile_pool(name="p1", bufs=8) as p1,
                tc.tile_pool(name="p1ps", bufs=4, space="PSUM") as p1ps,
                tc.tile_pool(name="p1y", bufs=8) as p1y,
            ):
                for i in range(NPAD // SC):
                    ez_t = p1.tile([128, 2, SC], bf16, tag="ez")
                    nc.sync.dma_start(out=ez_t[:], in_=ezt_i[i])
                    s_ps = p1ps.tile([1, SC], f32, tag="sps")
                    nc.tensor.matmul(out=s_ps[:], lhsT=w_sb[:, 0:1], rhs=ez_t[:, 0, :],
                                     start=True, stop=False)
                    nc.tensor.matmul(out=s_ps[:], lhsT=w_sb[:, 1:2], rhs=ez_t[:, 1, :],
                                     start=False, stop=True)
                    s_row = p1y.tile([1, SC], f32, tag="srow")
                    if i % 2 == 0:
                        nc.scalar.copy(out=s_row[:], in_=s_ps[:])
                    else:
                        nc.vector.tensor_copy(out=s_row[:], in_=s_ps[:])
                    # store via gpsimd so the wait on the copy doesn't block
                    # the SP sequencer from issuing the next ez load
                    nc.gpsimd.dma_start(
                        out=y_d[i * SC:(i + 1) * SC].rearrange("(a b) -> a b", a=1),
                        in_=s_row[:])

            # ---------------- phase 2: segment machinery ----------------
            with (
                tc.tile_pool(name="scal", bufs=1) as sp,
                tc.tile_pool(name="sps2", bufs=1, space="PSUM") as sps,
            ):
                # collapse the 34 phase-1 DMA-queue waits into one barrier —
                # a DMA descriptor only carries a few sync-wait slots.
                tc.strict_bb_all_engine_barrier()

                y1 = sp.tile([128, C1], f32)
                nc.sync.dma_start(out=y1[:], in_=y_d[0:N1].rearrange("(p c) -> p c", c=C1))
                y2 = sp.tile([P2, C1], f32)
                nc.sync.dma_start(out=y2[:], in_=y_d[N1:NPAD].rearrange("(p c) -> p c", c=C1))
                # softplus(s) = ln(exp(s) + 1): the ACT tables in this toolchain
                # have no softplus entry, but ln+exp share one table set.
                nc.scalar.activation(out=y1[:], in_=y1[:], func=AF.Exp)
                nc.scalar.activation(out=y2[:], in_=y2[:], func=AF.Exp)
                nc.scalar.activation(out=y1[:], in_=y1[:], func=AF.Ln, bias=1.0)
                nc.scalar.activation(out=y2[:], in_=y2[:], func=AF.Ln, bias=1.0)

                def cumsum_pair(t1, t2, name):
                    # inclusive cumsum over atoms laid out [128,C1] then [P2,C1]
                    z1 = sp.tile([128, C1], f32, tag=name + "z1")
                    nc.vector.tensor_tensor_scan(out=z1[:], data0=t1[:], data1=t1[:],
                                                 initial=0.0, op0=OP.add, op1=OP.bypass)
                    z2 = sp.tile([P2, C1], f32, tag=name + "z2")
                    nc.vector.tensor_tensor_scan(out=z2[:], data0=t2[:], data1=t2[:],
                                                 initial=0.0, op0=OP.add, op1=OP.bypass)
                    c1_ps = sps.tile([128, 1], f32, tag=name + "c1")
                    nc.tensor.matmul(out=c1_ps[:], lhsT=ltri_sb[:], rhs=z1[:, C1 - 1:C1],
                                     start=True, stop=True)
                    c2_ps = sps.tile([P2, 1], f32, tag=name + "c2")
                    nc.tensor.matmul(out=c2_ps[:], lhsT=ones_m[:, 0:P2], rhs=z1[:, C1 - 1:C1],
                                     start=True, stop=False)
                    nc.tensor.matmul(out=c2_ps[:], lhsT=ltri_sb[0:P2, 0:P2], rhs=z2[:, C1 - 1:C1],
                                     start=False, stop=True)
                    c1s = sp.tile([128, 1], f32, tag=name + "c1s")
                    nc.vector.tensor_copy(out=c1s[:], in_=c1_ps[:])
                    c2s = sp.tile([P2, 1], f32, tag=name + "c2s")
                    nc.vector.tensor_copy(out=c2s[:], in_=c2_ps[:])
                    zf1 = sp.tile([128, C1], f32, tag=name + "zf1")
                    nc.vector.tensor_scalar_add(out=zf1[:], in0=z1[:], scalar1=c1s[:])
                    zf2 = sp.tile([P2, C1], f32, tag=name + "zf2")
                    nc.vector.tensor_scalar_add(out=zf2[:], in0=z2[:], scalar1=c2s[:])
                    return zf1, zf2

                zf1, zf2 = cumsum_pair(y1, y2, "zy")
                nc.sync.dma_start(out=z_d[1:1 + N1].rearrange("(p c) -> p c", c=C1), in_=zf1[:])
                nc.sync.dma_start(out=z_d[1 + N1:1 + NPAD].rearrange("(p c) -> p c", c=C1), in_=zf2[:])

                import concourse.bass as bass_mod
                zdv = z_d[:].rearrange("(n o) -> n o", o=1)
                zp = sp.tile([128, GJ], f32)
                zm = sp.tile([128, GJ], f32)
                for j in range(GJ):
                    nc.gpsimd.indirect_dma_start(
                        out=zp[:, j:j + 1], out_offset=None, in_=zdv,
                        in_offset=bass_mod.IndirectOffsetOnAxis(ap=posp_sb[:, j:j + 1], axis=0))
                    nc.gpsimd.indirect_dma_start(
                        out=zm[:, j:j + 1], out_offset=None, in_=zdv,
                        in_offset=bass_mod.IndirectOffsetOnAxis(ap=posm_sb[:, j:j + 1], axis=0))

                den = sp.tile([128, GJ], f32)
                nc.vector.tensor_sub(den[:], zm[:], zp[:])
                nc.vector.tensor_scalar_max(out=den[:], in0=den[:], scalar1=1e-30)
                rec = sp.tile([128, GJ], f32)
                nc.vector.reciprocal(out=rec[:], in_=den[:])
                val = sp.tile([128, GJ], f32)
                nc.vector.tensor_mul(val[:], rec[:], psi_sb[:])

                dpv = dp_d[:].rearrange("(n o) -> n o", o=1)
                dmv = dm_d[:].rearrange("(n o) -> n o", o=1)
                for j in range(GJ):
                    nc.gpsimd.indirect_dma_start(
                        out=dpv, out_offset=bass_mod.IndirectOffsetOnAxis(ap=posp_sb[:, j:j + 1], axis=0),
                        in_=val[:, j:j + 1], in_offset=None)
                    nc.gpsimd.indirect_dma_start(
                        out=dmv, out_offset=bass_mod.IndirectOffsetOnAxis(ap=posm_sb[:, j:j + 1], axis=0),
                        in_=val[:, j:j + 1], in_offset=None)

                dd1 = sp.tile([128, C1], f32)
                dd2 = sp.tile([P2, C1], f32)
                tmp1 = sp.tile([128, C1], f32)
                tmp2 = sp.tile([P2, C1], f32)
                nc.sync.dma_start(out=dd1[:], in_=dp_d[0:N1].rearrange("(p c) -> p c", c=C1))
                nc.sync.dma_start(out=dd2[:], in_=dp_d[N1:NPAD].rearrange("(p c) -> p c", c=C1))
                nc.sync.dma_start(out=tmp1[:], in_=dm_d[0:N1].rearrange("(p c) -> p c", c=C1))
                nc.sync.dma_start(out=tmp2[:], in_=dm_d[N1:NPAD].rearrange("(p c) -> p c", c=C1))
                nc.vector.tensor_sub(dd1[:], dd1[:], tmp1[:])
                nc.vector.tensor_sub(dd2[:], dd2[:], tmp2[:])

                ef1, ef2 = cumsum_pair(dd1, dd2, "zd")
                a1 = sp.tile([128, C1], f32)
                nc.vector.tensor_mul(a1[:], y1[:], ef1[:])
                a2 = sp.tile([P2, C1], f32)
                nc.vector.tensor_mul(a2[:], y2[:], ef2[:])

                nc.sync.dma_start(out=a_d[0:N1].rearrange("(p c) -> p c", c=C1), in_=a1[:])
                nc.sync.dma_start(out=a_d[N1:NPAD].rearrange("(p c) -> p c", c=C1), in_=a2[:])
                # bf16 copy of a for the phase-3 partition-broadcast DMA
                ab1 = sp.tile([128, C1], bf16)
                nc.vector.tensor_copy(out=ab1[:], in_=a1[:])
                ab2 = sp.tile([P2, C1], bf16)
                nc.vector.tensor_copy(out=ab2[:], in_=a2[:])
                nc.sync.dma_start(out=ab_d[0:N1].rearrange("(p c) -> p c", c=C1), in_=ab1[:])
                nc.sync.dma_start(out=ab_d[N1:NPAD].rearrange("(p c) -> p c", c=C1), in_=ab2[:])

                art1 = sp.tile([C1, 128], f32)
                nc.sync.dma_start(out=art1[:], in_=a_d[0:N1].rearrange("(t q) -> t q", q=128))
                art2 = sp.tile([T2, 128], f32)
                nc.sync.dma_start(out=art2[:], in_=a_d[N1:NPAD].rearrange("(t q) -> t q", q=128))
                tp1 = sps.tile([128, C1], f32, tag="tp1")
                nc.tensor.transpose(out=tp1[:], in_=art1[:], identity=ident_sb[0:C1, 0:C1])
                tp2 = sps.tile([128, T2], f32, tag="tp2")
                nc.tensor.transpose(out=tp2[:], in_=art2[:], identity=ident_sb[0:T2, 0:T2])
                nc.vector.tensor_copy(out=a_colT[:, 0:C1], in_=tp1[:])
                nc.vector.tensor_copy(out=a_colT[:, C1:NT], in_=tp2[:])

            # ---------------- phase 3: MLP ----------------
            AB = A3 // 512 if A3 >= 512 else 1
            AS = A3 // AB                      # psum sub-width (<=512)
            with (
                tc.tile_pool(name="p3", bufs=2) as p3,
                tc.tile_pool(name="h1ps", bufs=2, space="PSUM") as h1ps,
                tc.tile_pool(name="ops", bufs=4, space="PSUM") as ops_,
                tc.tile_pool(name="p3o", bufs=2) as p3o,
            ):
                for b in range(NPAD // A3):
                    # broadcast a (bf16) down all 128 partitions via DMA
                    a_bc = p3.tile([128, A3], bf16, tag="abc")
                    a_sl = ab_d[b * A3:(b + 1) * A3]
                    a_sl_bc = bass.AP(
                        tensor=a_sl.tensor, offset=a_sl.offset,
                        ap=[[0, 128]] + [list(x) for x in a_sl.ap][-1:])
                    nc.sync.dma_start(out=a_bc[:], in_=a_sl_bc)
                    sx = p3.tile([128, 2, A3], bf16, tag="sx")
                    for k in range(2):
                        nc.scalar.activation(
                            out=sx[:, k, :], in_=a_bc[:], func=AF.Silu,
                            scale=vcol_sb[:, k:k + 1])
                    sh1 = p3.tile([128, 2, A3], bf16, tag="sh1")
                    for m in range(2):
                        h1 = h1ps.tile([128, AB, AS], f32, tag="h1")
                        for k in range(2):
                            for u in range(AB):
                                nc.tensor.matmul(
                                    out=h1[:, u, :],
                                    lhsT=w1_sb[:, k, m * 128:(m + 1) * 128],
                                    rhs=sx[:, k, u * AS:(u + 1) * AS],
                                    start=(k == 0), stop=(k == 1))
                        nc.scalar.activation(
                            out=sh1[:, m, :].rearrange("p (u n) -> p u n", u=AB),
                            in_=h1[:], func=AF.Silu)
                    osb = p3o.tile([128, A3 // 128, F], f32, tag="osb")
                    for t in range(A3 // 128):
                        o_ps = ops_.tile([128, F], f32, tag="ops")
                        nc.tensor.matmul(out=o_ps[:], lhsT=sh1[:, 0, t * 128:(t + 1) * 128],
                                         rhs=w2_sb[:, 0, :], start=True, stop=False)
                        nc.tensor.matmul(out=o_ps[:], lhsT=sh1[:, 1, t * 128:(t + 1) * 128],
                                         rhs=w2_sb[:, 1, :], start=False, stop=True)
                        gt = b * (A3 // 128) + t
                        nc.vector.scalar_tensor_tensor(
                            out=osb[:, t, :], in0=vrep_sb[:], scalar=a_colT[:, gt:gt + 1],
                            in1=o_ps[:], op0=OP.mult, op1=OP.add)
                    nc.sync.dma_start(
                        out=out_d[b * A3:(b + 1) * A3, :].rearrange("(t p) f -> p t f", p=128),
                        in_=osb[:])
    nc.finalize()
    return nc


def prep_core_inputs(cfg, core, eZ, psi, gb, w_bf, w1_bf, w2_bf, v0, ltri, ident):
    """Build the per-core input map (host-side sharding + packing)."""
    NPAD, N1 = cfg.NPAD, cfg.N1
    g0 = core * GPC
    s0, e0 = int(gb[g0]), int(gb[g0 + GPC])
    n_c = e0 - s0
    assert n_c <= NPAD, f"core {core}: {n_c} atoms > NPAD {NPAD}"

    ez_c = np.zeros((NPAD, F), np.float32)
    ez_c[:n_c] = eZ[s0:e0]
    NB1 = NPAD // cfg.SC
    ez_pack = np.ascontiguousarray(
        ez_c.reshape(NB1, cfg.SC, 2, 128).transpose(0, 3, 2, 1)).astype(BF16)

    gl = (gb[g0:g0 + GPC + 1] - s0).astype(np.int64)
    starts, ends = gl[:-1], gl[1:]
    nonempty = ends > starts
    stt = starts[nonempty]
    end_ = ends[nonempty]
    psi_ne = psi[g0:g0 + GPC][nonempty]
    K = len(stt)
    posp = np.zeros(GPC, np.int32)
    posm = np.zeros(GPC, np.int32)
    psig = np.zeros(GPC, np.float32)
    posp[:K] = stt
    posm[:K] = end_
    psig[:K] = psi_ne
    pad = np.arange(GPC - K, dtype=np.int32)
    posp[K:] = cfg.TRASH0 + pad
    posm[K:] = cfg.TRASH0 + pad

    def pack_g(x):
        return np.ascontiguousarray(x.reshape(GJ, 128).T)

    return {
        "ezt": ez_pack,
        "psig": pack_g(psig),
        "posp": pack_g(posp),
        "posm": pack_g(posm),
        "wv": np.ascontiguousarray(w_bf.reshape(2, 128).T),
        "w1": np.ascontiguousarray(w1_bf.reshape(2, 128, F).transpose(1, 0, 2)),
        "w2": np.ascontiguousarray(w2_bf.reshape(2, 128, F).transpose(1, 0, 2)),
        "vcol": np.ascontiguousarray(v0.reshape(2, 128).T.astype(np.float32)),
        "vrep": np.ascontiguousarray(np.broadcast_to(v0, (128, F)).astype(np.float32)),
        "ltri": ltri,
        "ident": ident,
    }, (s0, e0, n_c)


_NC_CACHE = {}


def kernel(atomic_numbers, psi, batch_segments, graph_mask, e_Z,
           W_q, k_table, v_table, W_res1, W_res2):
    from concourse.bass_utils import run_bass_kernel_spmd

    cfg = FULL
    psi = np.asarray(psi, np.float32)
    seg = np.asarray(batch_segments).astype(np.int64)
    eZ = np.asarray(e_Z, np.float32).reshape(-1, F)
    N = eZ.shape[0]
    assert N == N_FULL and len(psi) == G_FULL

    # fold weights: s = e_Z @ (W_q @ k0) / sqrt(F)   (psi // inf == 0 always)
    k0 = np.asarray(k_table, np.float32)[0]
    v0 = np.asarray(v_table, np.float32)[0]
    w = (np.asarray(W_q, np.float32) @ k0) * (1.0 / np.sqrt(F))
    w_bf = w.astype(BF16)
    w1_bf = np.asarray(W_res1, np.float32).astype(BF16)
    w2_bf = np.asarray(W_res2, np.float32).astype(BF16)
    ltri = np.triu(np.ones((128, 128), np.float32), 1)
    ident = np.eye(128, dtype=np.float32)

    gb = np.searchsorted(seg, np.arange(G_FULL + 1))

    in_maps, spans = [], []
    for c in range(NCORES):
        m, span = prep_core_inputs(cfg, c, eZ, psi, gb, w_bf, w1_bf, w2_bf,
                                   v0, ltri, ident)
        in_maps.append(m)
        spans.append(span)

    if "nc" not in _NC_CACHE:
        _NC_CACHE["nc"] = build_bass(cfg)
    nc = _NC_CACHE["nc"]

    trace = os.environ.get("KERNEL_TRACE", "") == "1"
    res = run_bass_kernel_spmd(nc, in_maps, core_ids=list(range(NCORES)),
                               trace=trace)
    if trace:
        kernel.last_exec_time_ns = res.exec_time_ns
        kernel.last_results = res

    out = np.empty((N, F), np.float32)
    for c in range(NCORES):
        s0, e0, n_c = spans[c]
        out[s0:e0] = res.results[c]["out"][:n_c]
    return out.reshape(N, 1, 1, F)
